# revision 3
# baseline (speedup 1.0000x reference)
"""EntropyAttentionHead Trainium2 kernel.

Per-(b,c) 256-bin histogram over [0,1] -> Shannon entropy -> broadcast to
the spatial map.  Pure data parallel over the 8 NeuronCores: 2048 (b,c)
pairs -> 256 per core.

Histogram strategy (per (b,c), 50176 pixels laid out as [128, 392] in SBUF):
  q  = floor(256*x) in {0..255}   (exact, via mod arithmetic)
  ih = q // 16, il = q % 16       (exact in bf16)
  Two 16-plane one-hot tensors (is_equal compares, DVE 4x mode), then the
  256-bin joint histogram is the 16x16 outer-product accumulation
      hist[h,l] = sum_p Hoh[p,h] * Loh[p,l]
  computed by the TensorEngine as 392 accumulating [K=128,M=16,N=16]
  matmuls into PSUM.  Entropy tail on ACT/DVE, per-core output broadcast.
"""

import os
import numpy as np

B, C, H, W = 16, 128, 224, 224
BINS = 256
NPIX = H * W            # 50176
P = 128
NCOLS = NPIX // P       # 392
NCORES = 8
BC_TOTAL = B * C        # 2048
NBC = BC_TOTAL // NCORES  # 256 per core


def build_nc(nbc=NBC, ncols=NCOLS):
    import concourse.bacc as bacc
    import concourse.bass as bass
    import concourse.tile as tile
    from concourse import mybir

    f32 = mybir.dt.float32
    bf16 = mybir.dt.bfloat16
    i32 = mybir.dt.int32
    OP = mybir.AluOpType
    AF = mybir.ActivationFunctionType

    nc = bacc.Bacc("TRN2", target_bir_lowering=False, debug=False)
    x_d = nc.dram_tensor("x", [nbc, P, ncols], f32, kind="ExternalInput").ap()
    o_d = nc.dram_tensor("o", [nbc, P, ncols], f32, kind="ExternalOutput").ap()

    inv_n = 1.0 / float(NPIX)

    with tile.TileContext(nc) as tc:
        with (
            tc.tile_pool(name="xin", bufs=3) as xin_p,
            tc.tile_pool(name="prep", bufs=3) as prep_p,
            tc.tile_pool(name="oh", bufs=2) as oh_p,
            tc.tile_pool(name="ps", bufs=4, space="PSUM") as ps_p,
            tc.tile_pool(name="tail", bufs=4) as tail_p,
            tc.tile_pool(name="fin", bufs=1) as fin_p,
            tc.tile_pool(name="dram", bufs=1, space="DRAM") as dram_p,
            tc.tile_pool(name="outp", bufs=3) as out_p,
            tc.tile_pool(name="pse", bufs=1, space="PSUM") as pse_p,
        ):
            ebuf = fin_p.tile([16, nbc], f32)
            eps16 = fin_p.tile([16, 1], f32)
            nc.vector.memset(eps16, 1e-10)
            ones16 = fin_p.tile([16, 1], f32)
            nc.vector.memset(ones16, 1.0)
            dz = fin_p.tile([P, ncols], f32)
            nc.vector.memset(dz, 0.0)

            for ibc in range(nbc):
                xt = xin_p.tile([P, ncols], f32, tag="xt")
                nc.sync.dma_start(out=xt, in_=x_d[ibc])

                # q = floor(256 x): r = round_i32(256x); q = r - (r > 256x)
                t = prep_p.tile([P, ncols], f32, tag="t")
                nc.vector.tensor_scalar(
                    out=t, in0=xt, scalar1=256.0, scalar2=None, op0=OP.mult)
                ri = prep_p.tile([P, ncols], i32, tag="ri")
                nc.vector.tensor_copy(out=ri, in_=t)
                r = prep_p.tile([P, ncols], f32, tag="r")
                nc.vector.tensor_copy(out=r, in_=ri)
                adj = prep_p.tile([P, ncols], f32, tag="adj")
                nc.vector.tensor_tensor(out=adj, in0=r, in1=t, op=OP.is_gt)
                q = prep_p.tile([P, ncols], bf16, tag="q")
                nc.vector.tensor_tensor(out=q, in0=r, in1=adj, op=OP.subtract)
                # ih = floor(q/16) same trick (bf16 exact); il = q - 16*ih
                u = prep_p.tile([P, ncols], bf16, tag="u")
                nc.vector.tensor_scalar(
                    out=u, in0=q, scalar1=0.0625, scalar2=None, op0=OP.mult)
                ui = prep_p.tile([P, ncols], i32, tag="ui")
                nc.vector.tensor_copy(out=ui, in_=u)
                r2 = prep_p.tile([P, ncols], bf16, tag="r2")
                nc.vector.tensor_copy(out=r2, in_=ui)
                adj2 = prep_p.tile([P, ncols], bf16, tag="adj2")
                nc.vector.tensor_tensor(out=adj2, in0=r2, in1=u, op=OP.is_gt)
                ih = prep_p.tile([P, ncols], bf16, tag="ih")
                nc.vector.tensor_tensor(out=ih, in0=r2, in1=adj2, op=OP.subtract)
                il = prep_p.tile([P, ncols], bf16, tag="il")
                nc.vector.scalar_tensor_tensor(
                    out=il, in0=ih, scalar=-16.0, in1=q,
                    op0=OP.mult, op1=OP.add)

                # one-hot planes: [128, 32, ncols]; 0..15 = ih planes, 16..31 = il
                oh = oh_p.tile([P, 32, ncols], bf16, tag="oh")
                for j in range(16):
                    nc.vector.tensor_scalar(
                        out=oh[:, j, :], in0=ih, scalar1=float(j),
                        scalar2=None, op0=OP.is_equal)
                    nc.vector.tensor_scalar(
                        out=oh[:, 16 + j, :], in0=il, scalar1=float(j),
                        scalar2=None, op0=OP.is_equal)

                # joint histogram: 392 accumulating matmuls
                ps = ps_p.tile([16, 16], f32, tag="ps")
                for n in range(ncols):
                    nc.tensor.matmul(
                        out=ps,
                        lhsT=oh[:, 0:16, n:n + 1],
                        rhs=oh[:, 16:32, n:n + 1],
                        start=(n == 0), stop=(n == ncols - 1))

                # entropy tail: sum p*ln(p + 1e-10), p = c/NPIX
                u = tail_p.tile([16, 16], f32, tag="u")
                nc.scalar.activation(
                    out=u, in_=ps, func=AF.Ln, bias=eps16, scale=inv_n)
                term = tail_p.tile([16, 16], f32, tag="term")
                nc.vector.scalar_tensor_tensor(
                    out=term, in0=ps, scalar=inv_n, in1=u,
                    op0=OP.mult, op1=OP.mult)
                nc.vector.tensor_reduce(
                    out=ebuf[:, ibc:ibc + 1], in_=term,
                    axis=mybir.AxisListType.XYZW, op=OP.add)

            # reduce over the 16 partitions with a ones-matmul, negate
            pse = pse_p.tile([1, nbc], f32)
            nc.tensor.matmul(out=pse, lhsT=ones16, rhs=ebuf, start=True, stop=True)
            esb = fin_p.tile([1, nbc], f32)
            nc.scalar.activation(out=esb, in_=pse, func=AF.Copy, scale=-1.0)

            # broadcast to 128 partitions via DRAM roundtrip
            edram = dram_p.tile([1, nbc], f32)
            nc.sync.dma_start(out=edram, in_=esb)
            e128 = fin_p.tile([P, nbc], f32)
            bcast = bass.AP(
                tensor=edram.tensor, offset=edram.offset,
                ap=[[0, P], list(edram.ap[-1])])
            nc.sync.dma_start(out=e128, in_=bcast)

            for ibc in range(nbc):
                ot = out_p.tile([P, ncols], f32, tag="ot")
                nc.scalar.activation(
                    out=ot, in_=dz, func=AF.Identity,
                    bias=e128[:, ibc:ibc + 1], scale=0.0)
                nc.sync.dma_start(out=o_d[ibc], in_=ot)

    nc.finalize()
    return nc


_NC_CACHE = {}


def _get_nc(nbc=NBC):
    if nbc not in _NC_CACHE:
        _NC_CACHE[nbc] = build_nc(nbc)
    return _NC_CACHE[nbc]


def run_sharded(x_r, nbc=NBC):
    """x_r: [ncores*nbc, P, NCOLS] float32 -> same-shape output."""
    from concourse.bass_utils import run_bass_kernel_spmd

    nc = _get_nc(nbc)
    ncores = x_r.shape[0] // nbc
    in_maps = [
        {"x": np.ascontiguousarray(x_r[i * nbc:(i + 1) * nbc])}
        for i in range(ncores)
    ]
    res = run_bass_kernel_spmd(nc, in_maps, core_ids=list(range(ncores)))
    out = np.concatenate([r["o"] for r in res.results], axis=0)
    return out


def kernel(x, bins):
    assert int(bins) == BINS
    x = np.asarray(x, dtype=np.float32)
    assert x.shape == (B, C, H, W), x.shape
    x_r = x.reshape(BC_TOTAL, P, NCOLS)
    out = run_sharded(x_r, NBC)
    return out.reshape(B, C, H, W).astype(np.float32)
